# revision 28
# baseline (speedup 1.0000x reference)
"""Trainium2 Bass kernel for MLA-style sparse (top-k) causal attention.

Strategy (8-way tensor parallel over heads, 2 heads per core):
  - The per-query top-32 gather is recast as dense masked attention:
    host builds a multiplicity mask MT[key, query] = count of `key` in
    topk_idx[query, :].  Then per head
        S^T  = (k @ q^T + kr @ qr^T) * scale          (dense, key-major)
        P    = MT * exp(S^T)
        out^T = v^T @ P,   r = 1^T @ P                (softmax denominator)
        out  = out^T / r
    which is exactly softmax-over-selected-logits with duplicate handling.
  - Down projections (x@Wqd, x@Wkvd, x@Wkr) are replicated on every core;
    up-projections are column-parallel per head; Wo is row-parallel and the
    8 partial (T, D) outputs are summed on the host.
  - All matmuls run in bf16 with fp32 PSUM accumulation.
"""

import math

import numpy as np
import ml_dtypes

import concourse.bass as bass
import concourse.mybir as mybir
import concourse.tile as tile
from concourse import bacc
from concourse.bass_utils import run_bass_kernel_spmd

F32 = mybir.dt.float32
BF16 = mybir.dt.bfloat16
AF = mybir.ActivationFunctionType

B, T, D = 1, 2048, 2048
H, HD = 16, 128
L = 512
RD = 64
K = 32
THETA = 10000.0
NCORES = 8
HL = H // NCORES          # heads per core = 2
CH = HL * HD              # head dims per core = 256
SCALE = 1.0 / math.sqrt(HD + RD)

NDT = D // 128            # 16 tiles over D
NLT = L // 128            # 4 tiles over L
NKT = T // 128            # 16 key tiles
NQT = T // 128            # 16 query tiles
QC = 1024                 # query-chunk (PSUM-bank budget)
NQC = T // QC             # 2 query chunks
TC = T // NCORES          # per-core token slice for sharded down-proj = 256
NBLK = NLT                # sharded down-proj blocks: 4x ckv (kr replicated)


def build_bass():
    nc = bacc.Bacc("TRN2", target_bir_lowering=False, debug=False, num_devices=NCORES)

    # ---- DRAM I/O (per-core values supplied via in_maps) ----
    d_xt = nc.dram_tensor("xt", [D, T], BF16, kind="ExternalInput")
    d_xtc = nc.dram_tensor("xtc", [D, TC], BF16, kind="ExternalInput")
    d_mt = nc.dram_tensor("mt", [T, T], BF16, kind="ExternalInput")
    d_cos = nc.dram_tensor("cos2", [128, T], BF16, kind="ExternalInput")
    d_ssin = nc.dram_tensor("ssin2", [128, T], BF16, kind="ExternalInput")
    d_wqd = nc.dram_tensor("wqd", [D, L], BF16, kind="ExternalInput")
    d_wkvd = nc.dram_tensor("wkvd", [D, L], BF16, kind="ExternalInput")
    d_wkr = nc.dram_tensor("wkr", [D, RD], BF16, kind="ExternalInput")
    d_wqu = nc.dram_tensor("wqu", [L, CH], BF16, kind="ExternalInput")
    d_wku = nc.dram_tensor("wku", [L, CH], BF16, kind="ExternalInput")
    d_wvu = nc.dram_tensor("wvu", [L, CH], BF16, kind="ExternalInput")
    d_wqr = nc.dram_tensor("wqr", [L, HL * RD], BF16, kind="ExternalInput")
    d_wo = nc.dram_tensor("wo", [HL, HD, D], BF16, kind="ExternalInput")
    d_y = nc.dram_tensor("y", [T, D], BF16, kind="ExternalOutput")
    d_rscr = nc.dram_tensor("r_scratch", [NQC * HL, QC], F32)  # internal

    with tile.TileContext(nc) as tc:
        with (
            tc.tile_pool(name="perm", bufs=1) as perm,
            tc.tile_pool(name="evict", bufs=1) as pe_pool,
        ):
            # ---- persistent tiles ----
            ql_sb = perm.tile([128, NLT, T], BF16)    # ql^T  (L, T)
            ckv_t = [perm.tile([128, T], BF16, name=f"ckv{i}") for i in range(NLT)]
            q_sb = perm.tile([128, HL, T], BF16)      # q^T per head (hd, T), scaled
            k_sb0 = perm.tile([128, T], BF16)         # k^T head 0 (hd, T)
            k_sb1 = perm.tile([128, T], BF16)         # k^T head 1 (hd, T)
            v_sb = perm.tile([128, NKT, CH], BF16)    # v natural (T, 256) tiled
            qr_sb = perm.tile([128, T], BF16)         # roped q rope dims, 2 heads stacked
            qr1_sb = perm.tile([64, T], BF16)         # head-1 rows rebased to partition 0
            krr_sb = perm.tile([64, T], BF16)         # roped shared k rope dims
            out_sb = perm.tile([128, HL, T], BF16)    # normalized attn out^T per head
            wo_sb = perm.tile([128, HL, D], BF16)
            ones_sb = perm.tile([128, 1], BF16)
            kr_bf = perm.tile([64, T], BF16)
            nc.vector.memset(ones_sb[:], 1.0)

            mc_cm = tc.tile_pool(name="mc", bufs=NKT)
            mc = mc_cm.__enter__()
            m_pre = {}
            bw_cm = tc.tile_pool(name="bw", bufs=1)
            bw = bw_cm.__enter__()
            cos_sb = bw.tile([128, T], BF16)
            ssin_sb = bw.tile([128, T], BF16)
            wqu_sb = bw.tile([128, NLT, CH], BF16)
            wku_sb = bw.tile([128, NLT, CH], BF16)
            wvu_sb = bw.tile([128, NLT, CH], BF16)
            wqr_sb = bw.tile([128, NLT, HL * RD], BF16)

            # ===== Phase A: ckv/kr T-sharded (+AllGather) overlapped with
            # ===== replicated ql down-projection =====
            with (
                tc.tile_pool(name="xa", bufs=1) as xa,
                tc.tile_pool(name="dra", bufs=1, space="DRAM") as dra,
                tc.tile_pool(name="psa", bufs=2, space="PSUM") as psa,
                tc.tile_pool(name="psq", bufs=2, space="PSUM") as psq,
            ):
                xc_sb = xa.tile([128, NDT, TC], BF16)
                stage = xa.tile([128, NBLK, TC], BF16)
                wkv_all = xa.tile([128, NDT, L + RD], BF16)
                cc_in = dra.tile([NBLK, 128, TC], BF16)
                cc_out = dra.tile([NCORES, NBLK, 128, TC], BF16, addr_space="Shared")
                for hh in range(4):
                    sl = slice(hh * 4, (hh + 1) * 4)
                    nc.sync.dma_start(
                        out=wkv_all[:, sl, :L],
                        in_=d_wkvd.ap().rearrange("(dt p) c -> p dt c", p=128)[:, sl, :],
                    )
                    nc.sync.dma_start(
                        out=xc_sb[:, sl, :],
                        in_=d_xtc.ap().rearrange("(dt p) t -> p dt t", p=128)[:, sl, :],
                    )
                nc.sync.dma_start(
                    out=wkv_all[:, :, L:],
                    in_=d_wkr.ap().rearrange("(dt p) c -> p dt c", p=128),
                )

                def down_pass_sharded(c0, cw, blk):
                    # stage[blk] = wkv[:, c0:c0+cw]^T @ x^T_slice over 16 D-tiles
                    ps = psa.tile([128, TC], F32, tag="psa")
                    for dt in range(NDT):
                        nc.tensor.matmul(
                            ps[:cw, :],
                            wkv_all[:, dt, c0 : c0 + cw],
                            xc_sb[:, dt, :],
                            start=(dt == 0),
                            stop=(dt == NDT - 1),
                        )
                    nc.vector.tensor_copy(stage[:cw, blk, :], ps[:cw, :])

                for lc in range(NLT):
                    down_pass_sharded(lc * 128, 128, lc)

                nc.gpsimd.dma_start(
                    out=cc_in.rearrange("blk p t -> p blk t"), in_=stage[:]
                )
                nc.gpsimd.collective_compute(
                    "AllGather",
                    mybir.AluOpType.bypass,
                    replica_groups=[list(range(NCORES))],
                    ins=[cc_in.opt()],
                    outs=[cc_out.opt()],
                )
                for lt in range(NLT):
                    nc.gpsimd.dma_start(
                        out=ckv_t[lt][:, :].rearrange("p (c t) -> p c t", c=NCORES),
                        in_=cc_out[:, lt, :, :].rearrange("c p t -> p c t"),
                    )


                # replicated ql = Wqd^T @ x^T over full T (overlaps the AllGather),
                # quarter-T waves, software-pipelined x loads; wqd resident once.
                wq_all = xa.tile([128, NDT, L], BF16)
                for lc in range(NLT):
                    nc.scalar.dma_start(
                        out=wq_all[:, :, lc * 128 : (lc + 1) * 128],
                        in_=d_wqd.ap()[:, lc * 128 : (lc + 1) * 128].rearrange(
                            "(dt p) c -> p dt c", p=128
                        ),
                    )
                QW = 256
                NW = T // QW

                def load_xq(tw):
                    x_q = xa.tile([128, NDT, QW], BF16, tag="xq", bufs=2)
                    nc.sync.dma_start(
                        out=x_q[:],
                        in_=d_xt.ap()[:, tw * QW : (tw + 1) * QW].rearrange(
                            "(dt p) t -> p dt t", p=128
                        ),
                    )
                    return x_q

                xq_next = load_xq(0)
                for tw in range(NW):
                    x_q = xq_next
                    if tw + 1 < NW:
                        xq_next = load_xq(tw + 1)
                    for lc in range(NLT):
                        ps = psq.tile([128, QW], F32, tag="psq")
                        for dt in range(NDT):
                            nc.tensor.matmul(
                                ps[:, :],
                                wq_all[:, dt, lc * 128 : (lc + 1) * 128],
                                x_q[:, dt, :],
                                start=(dt == 0),
                                stop=(dt == NDT - 1),
                            )
                        nc.vector.tensor_copy(
                            ql_sb[:, lc, tw * QW : (tw + 1) * QW], ps[:, :]
                        )
                    psk = psq.tile([128, QW], F32, tag="psq")
                    for dt in range(NDT):
                        nc.tensor.matmul(
                            psk[:RD, :],
                            wkv_all[:, dt, L : L + RD],
                            x_q[:, dt, :],
                            start=(dt == 0),
                            stop=(dt == NDT - 1),
                        )
                    nc.vector.tensor_copy(
                        kr_bf[:, tw * QW : (tw + 1) * QW], psk[:RD, :]
                    )

                nc.scalar.dma_start(out=cos_sb[:], in_=d_cos[:, :])
                nc.scalar.dma_start(out=ssin_sb[:], in_=d_ssin[:, :])
                for w_sb, w_dram in (
                    (wku_sb, d_wku),
                    (wvu_sb, d_wvu),
                ):
                    nc.scalar.dma_start(
                        out=w_sb[:],
                        in_=w_dram.ap().rearrange("(lt p) c -> p lt c", p=128),
                    )
                nc.scalar.dma_start(out=wo_sb[:, 0, :], in_=d_wo[0, :, :])
                nc.scalar.dma_start(out=wo_sb[:, 1, :], in_=d_wo[1, :, :])
                for kt in range(8):
                    m_t = mc.tile([128, QC], BF16, tag="mc")
                    nc.sync.dma_start(
                        out=m_t[:], in_=d_mt[kt * 128 : (kt + 1) * 128, 0:QC]
                    )
                    m_pre[kt] = m_t

            # ============= Phase B: up projections + rope =============
                with (
                    tc.tile_pool(name="bp", bufs=1) as bp,
                    tc.tile_pool(name="psb", bufs=2, space="PSUM") as psb,
                ):
                    cos_sb = bp.tile([128, T], F32)
                    ssin_sb = bp.tile([128, T], F32)
                    wqu_sb = bp.tile([128, NLT, CH], BF16)
                    wku_sb = bp.tile([128, NLT, CH], BF16)
                    wvu_sb = bp.tile([128, NLT, CH], BF16)
                    wqr_sb = bp.tile([128, NLT, HL * RD], BF16)
                    nc.sync.dma_start(out=cos_sb[:], in_=d_cos[:, :])
                    nc.sync.dma_start(out=ssin_sb[:], in_=d_ssin[:, :])
                    for w_sb, w_dram in (
                        (wqu_sb, d_wqu),
                        (wku_sb, d_wku),
                        (wvu_sb, d_wvu),
                        (wqr_sb, d_wqr),
                    ):
                        nc.sync.dma_start(
                            out=w_sb[:],
                            in_=w_dram.ap().rearrange("(lt p) c -> p lt c", p=128),
                        )

                    def up_pass(w_sb, c0, cw, acts_sb, out_cb):
                        ps = psb.tile([128, T], F32, tag="psb")
                        for lt in range(NLT):
                            for j in range(4):
                                nc.tensor.matmul(
                                    ps[:cw, j * 512 : (j + 1) * 512],
                                    w_sb[:, lt, c0 : c0 + cw],
                                    acts_sb[:, lt, j * 512 : (j + 1) * 512],
                                    start=(lt == 0),
                                    stop=(lt == NLT - 1),
                                )
                        out_cb(ps)

                    for h in range(HL):
                        up_pass(
                            wqu_sb, h * HD, HD, ql_list,
                            lambda ps, h=h: nc.scalar.activation(
                                q_sb[:, h, :], ps[:, :], AF.Copy, scale=SCALE
                            ),
                        )
                        up_pass(
                            wku_sb, h * HD, HD, ckv_t,
                            lambda ps, h=h: nc.scalar.activation(
                                k_sb[:, h, :], ps[:, :], AF.Copy
                            ),
                        )

                    # ---- rope on q rope-dims (both heads stacked: 2x64 rows) ----
                    qr_f32 = bp.tile([128, T], F32)
                    qr_shift = bp.tile([128, T], F32)
                    qr_tmp = bp.tile([128, T], F32)
                    up_pass(
                        wqr_sb, 0, HL * RD, ql_list,
                        lambda ps: nc.scalar.activation(
                            qr_f32[:, :], ps[:, :], AF.Copy, scale=SCALE
                        ),
                    )
                    for blk in range(2 * HL):  # 4 blocks of 32 partitions
                        b0 = blk * 32
                        src = b0 + 32 if blk % 2 == 0 else b0 - 32
                        nc.sync.dma_start(
                            out=qr_shift[b0 : b0 + 32, :], in_=qr_f32[src : src + 32, :]
                        )
                    nc.vector.tensor_mul(qr_tmp[:, :], qr_f32[:, :], cos_sb[:, :])
                    nc.vector.tensor_mul(qr_shift[:, :], qr_shift[:, :], ssin_sb[:, :])
                    nc.vector.tensor_add(qr_sb[:, :], qr_tmp[:, :], qr_shift[:, :])
                    nc.sync.dma_start(out=qr1_sb[:, :], in_=qr_sb[64:128, :])

                    # ---- rope on shared k rope-dims (64 rows) ----
                    kr_shift = bp.tile([64, T], F32)
                    kr_tmp = bp.tile([64, T], F32)
                    nc.sync.dma_start(out=kr_shift[0:32, :], in_=kr_bf[32:64, :])
                    nc.sync.dma_start(out=kr_shift[32:64, :], in_=kr_bf[0:32, :])
                    nc.vector.tensor_mul(kr_tmp[:, :], kr_bf[:, :], cos_sb[:64, :])
                    nc.vector.tensor_mul(kr_shift[:, :], kr_shift[:, :], ssin_sb[:64, :])
                    nc.vector.tensor_add(krr_sb[:, :], kr_tmp[:, :], kr_shift[:, :])

                    # ---- v in natural (T, 256) layout ----
                    with tc.tile_pool(name="psv", bufs=6, space="PSUM") as psv:
                        for tt in range(NKT):
                            ps = psv.tile([128, CH], F32, tag="psv")
                            for lt in range(NLT):
                                nc.tensor.matmul(
                                    ps[:, :],
                                    ckv_sb[:, lt, tt * 128 : (tt + 1) * 128],
                                    wvu_sb[:, lt, :],
                                    start=(lt == 0),
                                    stop=(lt == NLT - 1),
                                )
                            nc.vector.tensor_copy(v_sb[:, tt, :], ps[:, :])

            # ================= Phase C: masked attention =================
            with (
                tc.tile_pool(name="ec", bufs=4) as ec,
                tc.tile_pool(name="pc", bufs=NKT) as pc,
                tc.tile_pool(name="rc", bufs=2) as rc,
                tc.tile_pool(name="ps_st", bufs=2, space="PSUM") as ps_st,
                tc.tile_pool(name="ps_o", bufs=1, space="PSUM") as ps_o,
                tc.tile_pool(name="ps_r", bufs=1, space="PSUM") as ps_r,
            ):
                for qc in range(NQC):
                    q0 = qc * QC
                    m_tiles = dict(m_pre) if qc == 0 else {}
                    for h in range(HL):
                        qr_h = qr_sb[0:64, :] if h == 0 else qr1_sb[:, :]
                        p_tiles = []
                        for kt in range(NKT):
                            if h == 0:
                                m_t = mc.tile([128, QC], BF16, tag="mc")
                                nc.gpsimd.dma_start(
                                    out=m_t[:],
                                    in_=d_mt[kt * 128 : (kt + 1) * 128, q0 : q0 + QC],
                                )
                                m_tiles[kt] = m_t
                            ps = ps_st.tile([128, QC], F32, tag="ps_st")
                            for j in range(QC // 512):
                                s = slice(j * 512, (j + 1) * 512)
                                qs = slice(q0 + j * 512, q0 + (j + 1) * 512)
                                nc.tensor.matmul(
                                    ps[:, s],
                                    (k_sb0 if h == 0 else k_sb1)[
                                        :, kt * 128 : (kt + 1) * 128
                                    ],
                                    q_sb[:, h, qs],
                                    start=True,
                                    stop=False,
                                )
                            for j in range(QC // 512):
                                s = slice(j * 512, (j + 1) * 512)
                                qs = slice(q0 + j * 512, q0 + (j + 1) * 512)
                                nc.tensor.matmul(
                                    ps[:, s],
                                    krr_sb[:, kt * 128 : (kt + 1) * 128],
                                    qr_h[:, qs],
                                    start=False,
                                    stop=True,
                                )
                            e_t = ec.tile([128, QC], BF16, tag="ec")
                            nc.scalar.activation(e_t[:], ps[:, :], AF.Exp)
                            p_t = pc.tile([128, QC], BF16, tag="pc")
                            nc.vector.tensor_mul(p_t[:], e_t[:], m_tiles[kt][:])
                            p_tiles.append(p_t)

                        # softmax denominator r = 1^T @ P
                        rps = ps_r.tile([1, QC], F32, tag="ps_r")
                        for kt in range(NKT):
                            for j in range(QC // 512):
                                s = slice(j * 512, (j + 1) * 512)
                                nc.tensor.matmul(
                                    rps[:, s],
                                    ones_sb[:, :],
                                    p_tiles[kt][:, s],
                                    start=(kt == 0),
                                    stop=(kt == NKT - 1),
                                )
                        r_row = rc.tile([1, QC], F32, tag="r_row")
                        nc.vector.reciprocal(r_row[:], rps[:, :])
                        scr = d_rscr[qc * HL + h : qc * HL + h + 1, :]
                        nc.gpsimd.dma_start(out=scr, in_=r_row[:])
                        r_full = rc.tile([128, QC], F32, tag="r_full")
                        nc.gpsimd.dma_start(out=r_full[:], in_=scr.to_broadcast([128, QC]))

                        # out^T = v^T @ P
                        ops = ps_o.tile([128, QC], F32, tag="ps_o")
                        for kt in range(NKT):
                            for j in range(QC // 512):
                                s = slice(j * 512, (j + 1) * 512)
                                nc.tensor.matmul(
                                    ops[:, s],
                                    v_sb[:, kt, h * HD : (h + 1) * HD],
                                    p_tiles[kt][:, s],
                                    start=(kt == 0),
                                    stop=(kt == NKT - 1),
                                )
                        nc.vector.tensor_mul(
                            out_sb[:, h, q0 : q0 + QC], ops[:, :], r_full[:]
                        )

            # ================= Phase D: output projection =================
            with (
                tc.tile_pool(name="yd", bufs=4) as yd,
                tc.tile_pool(name="ps_y", bufs=2, space="PSUM") as ps_y,
            ):
                for qt in range(NQT):
                    ps = ps_y.tile([128, D], F32, tag="ps_y")
                    for h in range(HL):
                        for j in range(4):
                            nc.tensor.matmul(
                                ps[:, j * 512 : (j + 1) * 512],
                                out_sb[:, h, qt * 128 : (qt + 1) * 128],
                                wo_sb[:, h, j * 512 : (j + 1) * 512],
                                start=(h == 0),
                                stop=(h == HL - 1),
                            )
                    y_t = yd.tile([128, D], BF16, tag="yd")
                    nc.vector.tensor_copy(y_t[:], ps[:, :])
                    eng = nc.sync if qt % 2 == 0 else nc.scalar
                    eng.dma_start(
                        out=d_y[qt * 128 : (qt + 1) * 128, :], in_=y_t[:]
                    )
            bw_cm.__exit__(None, None, None)
            mc_cm.__exit__(None, None, None)

    nc.compile()
    return nc


_NC_CACHE = None


def _get_nc():
    global _NC_CACHE
    if _NC_CACHE is None:
        _NC_CACHE = build_bass()
    return _NC_CACHE


def _bf16(a):
    return np.asarray(a, dtype=np.float32).astype(ml_dtypes.bfloat16)


def make_in_maps(x, Wqd, Wqu, Wqr, Wkvd, Wku, Wvu, Wkr, Wo, topk_idx):
    x2 = np.asarray(x, dtype=np.float32).reshape(T, D)
    xt = np.ascontiguousarray(x2.T).astype(ml_dtypes.bfloat16)

    idx = np.asarray(topk_idx).reshape(T, K).astype(np.int64)
    mt = np.zeros((T, T), dtype=np.int16)
    np.add.at(mt, (idx.reshape(-1), np.repeat(np.arange(T), K)), 1)
    mt = mt.astype(ml_dtypes.bfloat16)

    # rope tables, transposed, with rotate-half folded into a signed sin table
    freqs = (
        1.0 / (THETA ** (np.arange(0, RD, 2, dtype=np.float32) / RD))
    ).astype(np.float32)
    ang = np.outer(np.arange(T, dtype=np.float32), freqs)       # (T, RD/2)
    cos_t = np.concatenate([np.cos(ang), np.cos(ang)], axis=-1).T  # (RD, T)
    sin_t = np.concatenate([np.sin(ang), np.sin(ang)], axis=-1).T
    ssin_t = np.concatenate([-sin_t[: RD // 2], sin_t[RD // 2 :]], axis=0)
    cos2 = np.ascontiguousarray(
        np.concatenate([cos_t, cos_t], axis=0)
    ).astype(ml_dtypes.bfloat16)    # (128, T)
    ssin2 = np.ascontiguousarray(
        np.concatenate([ssin_t, ssin_t], axis=0)
    ).astype(ml_dtypes.bfloat16)

    wqd32 = np.asarray(Wqd, dtype=np.float32)
    wqe = _bf16(wqd32 @ np.asarray(Wqu, dtype=np.float32))
    wqre = _bf16(wqd32 @ np.asarray(Wqr, dtype=np.float32))
    wkvd = _bf16(Wkvd)
    wkr = _bf16(Wkr)
    wku = _bf16(Wku)
    wvu = _bf16(Wvu)
    wo = _bf16(Wo).reshape(H, HD, D)

    in_maps = []
    for c in range(NCORES):
        cs, ce = c * CH, (c + 1) * CH
        in_maps.append(
            {
                "xt": xt,
                "xtc": np.ascontiguousarray(xt[:, c * TC : (c + 1) * TC]),
                "mt": mt,
                "cos2": cos2,
                "ssin2": ssin2,
                "wkvd": wkvd,
                "wkr": wkr,
                "wqe": np.ascontiguousarray(wqe[:, cs:ce]),
                "wqre": np.ascontiguousarray(
                    wqre[:, c * HL * RD : (c + 1) * HL * RD]
                ),
                "wku": np.ascontiguousarray(wku[:, cs:ce]),
                "wvu": np.ascontiguousarray(wvu[:, cs:ce]),
                "wo": np.ascontiguousarray(wo[c * HL : (c + 1) * HL]),
            }
        )
    return in_maps


def kernel(x, Wqd, Wqu, Wqr, Wkvd, Wku, Wvu, Wkr, Wo, topk_idx):
    nc = _get_nc()
    in_maps = make_in_maps(
        x, Wqd, Wqu, Wqr, Wkvd, Wku, Wvu, Wkr, Wo, topk_idx
    )
    res = run_bass_kernel_spmd(nc, in_maps, core_ids=list(range(NCORES)))
    y = np.zeros((T, D), dtype=np.float32)
    for c in range(NCORES):
        y += res.results[c]["y"].astype(np.float32)
    return y.reshape(B, T, D)


if __name__ == "__main__":
    rng = np.random.default_rng(0)
    inputs = {
        "x": rng.standard_normal((B, T, D)).astype(np.float32),
        "Wqd": (rng.standard_normal((D, L)) / math.sqrt(D)).astype(np.float32),
        "Wqu": (rng.standard_normal((L, D)) / math.sqrt(L)).astype(np.float32),
        "Wqr": (rng.standard_normal((L, H * RD)) / math.sqrt(L)).astype(np.float32),
        "Wkvd": (rng.standard_normal((D, L)) / math.sqrt(D)).astype(np.float32),
        "Wku": (rng.standard_normal((L, D)) / math.sqrt(L)).astype(np.float32),
        "Wvu": (rng.standard_normal((L, D)) / math.sqrt(L)).astype(np.float32),
        "Wkr": (rng.standard_normal((D, RD)) / math.sqrt(D)).astype(np.float32),
        "Wo": (rng.standard_normal((D, D)) / math.sqrt(D)).astype(np.float32),
        "topk_idx": rng.integers(0, T, (B, T, K)).astype(np.int32),
    }
    out = kernel(**inputs)
    print("kernel output", out.shape, out.dtype, float(np.abs(out).mean()))
